# revision 23
# baseline (speedup 1.0000x reference)
"""Trainium2 Bass kernel for nn_CircuitModel (soft sequential XOR circuit).

Math: with u = 1 - 2*s (s = register value), soft-XOR becomes a pure product:
    u_new = u_a * u_b,   u_x = -tanh(2*clip(p, -2, 2))
Magnitudes and signs decouple and both evolve *linearly*, so the whole
64-step scan unrolls at build time (wa/wb known when kernel() is called):
    log|u_final[j]| = sum_k C[j,k] * ln(tanh(2|p_k|))   (k = used (i,t) cell)
    sign(u_final[j]) = (-1)^( sum_k C[j,k] * [p_k > 0]  mod 2 )
    (parity of C.v == parity of D.v for D = C mod 2)
Registers whose tree hits the t=0 init state (u=0) or has leaf count
>= 4096 (magnitude underflow) output exactly 0.5.

Sharding: pure batch-parallel, 512 batch per core. The only per-exec
input is the signed bf16 value of each used (i,t) cell:
    ap = p as bf16 [128, nchunks, 512]   (a pure re-layout of P's numbers)
The wiring matrices cw (counts, bf16) and dw (parity, fp8) are baked into
the NEFF as Const tensors (loaded to device DRAM once at model load, not
per execution). Signs v = [p > 0] and |t| are computed on device (both on
DVE), so neither is transferred. Output is uint8 -- well inside the 2e-2
gate (see decode note below).

Device per core / rep:
  phase A (ACT table exp_and_others):
    t = tanh(2*ap) [ACT, signed]
    v = [ap > 0] fp8 {0,1} [DVE]        ta = max(-t, t) [DVE]
    (both on DVE: GpSimd's fp8-out tensor_scalar traps to a ~20us/half
     software handler on real HW)
    Y = D^T v: fp8 DoubleRow matmuls (2 chunks/pass; counts exact in f32)
    parity of Y via round-to-nearest magic (DVE):
        g = Y/2 + 2^23; h = g - 2^23 (= RN(Y/2)); d = Y/2 - h in {0,+-1/2}
        b = [d != 0] - 1/2  in {+-1/2} (bf16 exact)
  phase B (ACT table natural_log_exp_and_others):
    l = ln(ta) [ACT]; L = C^T l: bf16 matmuls
    e = exp(L) [ACT]; y = b*e [DVE]; q = uint8(y*254 + 128) [DVE] -> D2H
    (phase-B tail is per-m-tile: each 128-register tile's exp/y/q/out-DMA
     pipelines behind its last matmul)
Host decodes s = (q - 128)/254 + 0.5; the device's float->uint8 cast is
round-to-nearest-even with saturation (probed on HW), so quantization
error is <= 1/508.
"""

import sys
from contextlib import ExitStack

import numpy as np

sys.path.insert(0, "/opt/trn_rl_repo")

import concourse.mybir as mybir  # noqa: E402
import concourse.tile as tile  # noqa: E402
from concourse import bacc, bass_utils  # noqa: E402

N_IN = 256
N_REG = 256
T = 64
B = 4096
NCORES = 8
BL = B // NCORES  # 512 batch per core
W_CAP = 4096  # leaf-count threshold beyond which u underflows to 0 -> s = 0.5

AF = mybir.ActivationFunctionType
ALU = mybir.AluOpType
MAGIC = float(1 << 23)  # 2^23: fp32 round-to-nearest-integer magic


class _Bacc(bacc.Bacc):
    """Bacc whose ACT-table chooser never picks the exp-less 'natural_log'
    set: Ln then resolves to 'natural_log_exp_and_others', so the Ln pass
    and the final Exp share one table load instead of two."""

    def insert_act_table_loads(self):
        import bass_rust
        from concourse.hw_specs import get_activation_tables

        has_activation = any(
            isinstance(i, mybir.InstActivation)
            for b in self.main_func.blocks
            for i in b.instructions
        )
        if not has_activation:
            return
        tables = [
            (name, (set() if name == "natural_log" else funcs))
            for name, funcs in get_activation_tables(self.m.arch).items()
        ]
        bass_rust.insert_act_table_loads(self, tables)


def _unroll(wa, wb):
    """Exact symbolic unroll of the 64-step recurrence.

    Returns (C counts int64 [N_REG, N_IN*T] saturating, Z bool: u == 0
    exactly because the tree reaches the init state)."""
    NC = N_IN * T
    C = np.zeros((N_REG, NC), np.int64)
    Z = np.ones(N_REG, bool)
    wa = np.asarray(wa).astype(np.int64)
    wb = np.asarray(wb).astype(np.int64)
    for t in range(T):
        nC = np.zeros_like(C)
        nZ = np.zeros(N_REG, bool)
        for src in (wa, wb):
            is_x = src < N_IN
            xrows = np.nonzero(is_x)[0]
            nC[xrows, src[xrows] * T + t] += 1
            rrows = np.nonzero(~is_x)[0]
            ri = src[rrows] - N_IN
            nC[rrows] += C[ri]
            nZ[rrows] |= Z[ri]
        np.minimum(nC, 1 << 20, out=nC)
        C, Z = nC, nZ
    return C, Z


def _build_plan(wa, wb):
    C, Z = _unroll(wa, wb)
    W = C.sum(1)
    alive = (~Z) & (W < W_CAP)
    aidx = np.nonzero(alive)[0]
    A = int(len(aidx))
    if A == 0:
        return {"A": 0, "aidx": aidx}
    Ca = C[aidx]
    used = (Ca != 0).any(0)
    cols = np.nonzero(used)[0]  # flattened (i*T + t) indices of used cells
    ncols = int(len(cols))
    nchunks = (ncols + 127) // 128
    nslots = nchunks * 128
    # pad slots duplicate the first used cell; their C columns stay zero
    slot_cols = np.concatenate([cols, np.full(nslots - ncols, cols[0], np.int64)])
    slot_i = slot_cols // T
    slot_t = slot_cols % T

    n_mt = (A + 127) // 128
    Apad = n_mt * 128
    # lhsT layout [slot-in-chunk (K), chunk * Apad + alive-row (M)]
    cw = np.zeros((128, nchunks * Apad), np.float32)
    dw = np.zeros((128, nchunks * Apad), np.float32)
    for s in range(ncols):
        ci = slot_cols[s]
        c, k = divmod(s, 128)
        cw[k, c * Apad : c * Apad + A] = Ca[:, ci]
        dw[k, c * Apad : c * Apad + A] = Ca[:, ci] % 2
    return {
        "A": A,
        "aidx": aidx,
        "slot_i": slot_i,
        "slot_t": slot_t,
        "nchunks": nchunks,
        "n_mt": n_mt,
        "Apad": Apad,
        "cw": cw,
        "dw": dw,
    }


def _build_nc(plan, reps=1, loop=1):
    """reps: python-unrolled body repetitions (for slope benchmarking).
    loop: hardware For_i trip count around the body."""
    f32 = mybir.dt.float32
    bf16 = mybir.dt.bfloat16
    f8 = mybir.dt.float8e4
    u8 = mybir.dt.uint8
    DR = mybir.MatmulPerfMode.DoubleRow
    nchunks, n_mt, Apad = plan["nchunks"], plan["n_mt"], plan["Apad"]
    bfnp = mybir.dt.np(bf16)
    f8np = mybir.dt.np(f8)

    nc = _Bacc("TRN2", debug=False)
    # [slot-in-chunk, chunk, batch], packed on host while sharding (signed p)
    ap_d = nc.dram_tensor("ap_used", [128, nchunks, BL], bf16, kind="ExternalInput")
    # wiring weights: Const tensors inside the NEFF (no per-exec H2D)
    cw_d = nc.inline_tensor(plan["cw"].astype(bfnp), name="cw_const")
    dw_d = nc.inline_tensor(plan["dw"].astype(f8np), name="dw_const")
    out_d = nc.dram_tensor("outs", [Apad, BL], u8, kind="ExternalOutput")

    # split chunks in two halves so ACT starts while the 2nd DMA flies
    h0 = max(1, nchunks // 2)
    halves = [(0, h0), (h0, nchunks)] if nchunks > 1 else [(0, nchunks)]

    with tile.TileContext(nc) as tc, ExitStack() as ctx:
        pool = ctx.enter_context(tc.tile_pool(name="pool", bufs=1))
        tmp = ctx.enter_context(tc.tile_pool(name="tmp", bufs=2))
        mps = ctx.enter_context(tc.tile_pool(name="mps", bufs=1, space="PSUM"))

        # weights go over the ACT-engine HWDGE queue so the batch input's
        # DMAs (SP queue) aren't queued behind them
        cw_s = pool.tile([128, nchunks * Apad], bf16)
        nc.scalar.dma_start(cw_s[:], cw_d[:])
        dw_s = pool.tile([128, nchunks * Apad], f8)
        nc.scalar.dma_start(dw_s[:], dw_d[:])
        cwv = cw_s.rearrange("k (c a) -> k c a", a=Apad)
        dwv = dw_s.rearrange("k (c a) -> k c a", a=Apad)

        def body():
            for r in range(reps):
                i = r % 2
                apv_d = ap_d.rearrange("k c b -> k (c b)")
                ap_s = pool.tile(
                    [128, nchunks * BL], bf16, name=f"ap{i}", tag=f"ap{i}"
                )
                t_s = pool.tile([128, nchunks * BL], bf16, name=f"t{i}", tag=f"t{i}")
                ta_s = pool.tile(
                    [128, nchunks * BL], bf16, name=f"ta{i}", tag=f"ta{i}"
                )
                v_s = pool.tile([128, nchunks * BL], f8, name=f"v{i}", tag=f"v{i}")
                for c0, c1 in halves:
                    sl = slice(c0 * BL, c1 * BL)
                    nc.sync.dma_start(ap_s[:, sl], apv_d[:, sl])
                    # ---- phase A: tanh (table: exp_and_others) ----
                    nc.scalar.activation(t_s[:, sl], ap_s[:, sl], AF.Tanh, scale=2.0)
                    # signs from raw p. NOT on GpSimd: Pool's is_gt
                    # tensor_scalar traps to a ~20us/half software handler
                    # on real HW regardless of dtype (cost model: 2.2us).
                    # Interleaved per half so the long v op never blocks the
                    # next |t| (critical for Ln) in the DVE program order.
                    nc.vector.tensor_scalar(
                        v_s[:, sl], ap_s[:, sl], 0.0, None, ALU.is_gt
                    )
                    # |t| on DVE: max(-t, t)
                    nc.vector.scalar_tensor_tensor(
                        ta_s[:, sl], t_s[:, sl], -1.0, t_s[:, sl], ALU.mult, ALU.max
                    )
                vv = v_s.rearrange("k (c b) -> k c b", b=BL)
                tav = ta_s.rearrange("k (c b) -> k c b", b=BL)

                # ---- parity counts: Y = D^T v (fp8 DoubleRow, 2 chunks
                # per pass: halves the cold-clock PE serial chain vs bf16)
                Y_ps = mps.tile([128, n_mt * BL], f32, name=f"Yp{i}", tag=f"Yp{i}")
                npair = nchunks // 2
                for c in range(npair):
                    for mt in range(n_mt):
                        nc.tensor.matmul(
                            Y_ps[:, mt * BL : (mt + 1) * BL],
                            dwv[:, 2 * c : 2 * c + 2, mt * 128 : (mt + 1) * 128],
                            vv[:, 2 * c : 2 * c + 2, :],
                            start=(c == 0),
                            stop=(c == npair - 1 and nchunks % 2 == 0),
                            perf_mode=DR,
                        )
                if nchunks % 2:  # odd tail chunk: plain fp8 matmul
                    c = nchunks - 1
                    for mt in range(n_mt):
                        nc.tensor.matmul(
                            Y_ps[:, mt * BL : (mt + 1) * BL],
                            dwv[:, c, mt * 128 : (mt + 1) * 128],
                            vv[:, c, :],
                            start=(nchunks == 1),
                            stop=True,
                        )
                # parity: b = [Y odd] - 1/2, exact via fp32 magic round
                g_t = tmp.tile([128, n_mt * BL], f32, tag="p_g")
                nc.vector.tensor_scalar(g_t[:], Y_ps[:], 0.5, MAGIC, ALU.mult, ALU.add)
                h_t = tmp.tile([128, n_mt * BL], f32, tag="p_h")
                nc.vector.tensor_scalar(h_t[:], g_t[:], MAGIC, None, ALU.subtract)
                d_t = tmp.tile([128, n_mt * BL], bf16, tag="p_d")
                nc.vector.scalar_tensor_tensor(
                    d_t[:], Y_ps[:], 0.5, h_t[:], ALU.mult, ALU.subtract
                )
                b_t = pool.tile([128, n_mt * BL], bf16, name=f"b{i}", tag=f"b{i}")
                nc.vector.tensor_scalar(
                    b_t[:], d_t[:], 0.0, 0.5, ALU.not_equal, ALU.subtract
                )

                # ---- phase B: ln + counts matmul + exp (table: nat_log_exp)
                l_s = pool.tile([128, nchunks * BL], bf16, name=f"l{i}", tag=f"l{i}")
                L_ps = mps.tile([128, n_mt * BL], f32, name=f"Lp{i}", tag=f"Lp{i}")
                lv = l_s.rearrange("k (c b) -> k c b", b=BL)
                e_t = tmp.tile([128, n_mt * BL], bf16, tag="p_e")
                y_t = tmp.tile([128, n_mt * BL], bf16, tag="p_y")
                q_t = tmp.tile([128, n_mt * BL], u8, tag="p_q")
                for c0, c1 in halves[:-1]:
                    sl = slice(c0 * BL, c1 * BL)
                    nc.scalar.activation(l_s[:, sl], ta_s[:, sl], AF.Ln)
                    for c in range(c0, c1):
                        for mt in range(n_mt):
                            nc.tensor.matmul(
                                L_ps[:, mt * BL : (mt + 1) * BL],
                                cwv[:, c, mt * 128 : (mt + 1) * 128],
                                lv[:, c, :],
                                start=(c == 0),
                                stop=False,
                            )
                # last half mt-major: exp/y/q/out-DMA pipeline per m-tile
                c0, c1 = halves[-1]
                sl = slice(c0 * BL, c1 * BL)
                nc.scalar.activation(l_s[:, sl], ta_s[:, sl], AF.Ln)
                for mt in range(n_mt):
                    mb = slice(mt * BL, (mt + 1) * BL)
                    for c in range(c0, c1):
                        nc.tensor.matmul(
                            L_ps[:, mb],
                            cwv[:, c, mt * 128 : (mt + 1) * 128],
                            lv[:, c, :],
                            start=(c == 0 and c0 == 0),
                            stop=(c == nchunks - 1),
                        )
                    nc.scalar.activation(e_t[:, mb], L_ps[:, mb], AF.Exp)
                    nc.vector.tensor_tensor(y_t[:, mb], b_t[:, mb], e_t[:, mb], ALU.mult)
                    nc.vector.tensor_scalar(
                        q_t[:, mb], y_t[:, mb], 254.0, 128.0, ALU.mult, ALU.add
                    )
                    nc.sync.dma_start(
                        out_d[mt * 128 : (mt + 1) * 128, :], q_t[:, mb]
                    )

        if loop > 1:
            with tc.For_i(0, loop):
                body()
        else:
            body()

    nc.compile()
    return nc


_CACHE = {}


def _get_compiled(wa, wb):
    key = (np.asarray(wa).tobytes(), np.asarray(wb).tobytes())
    if key not in _CACHE:
        plan = _build_plan(wa, wb)
        nc = _build_nc(plan) if plan["A"] > 0 else None
        _CACHE[key] = (plan, nc)
    return _CACHE[key]


def make_in_maps(P, plan):
    """Pack per-core input: signed p of used cells, bf16 [128, nchunks, BL]."""
    bf = mybir.dt.np(mybir.dt.bfloat16)
    nchunks = plan["nchunks"]
    sel_all = P[plan["slot_i"], :, plan["slot_t"]]  # [nslots, B] f32
    sel_all = sel_all.reshape(nchunks, 128, B).transpose(1, 0, 2)  # [128,c,B]
    sel_all = sel_all.astype(bf)
    return [
        {"ap_used": np.ascontiguousarray(sel_all[:, :, c * BL : (c + 1) * BL])}
        for c in range(NCORES)
    ]


def run(P, wa, wb, trace=False):
    """Returns (out [B, N_REG] float32, BassKernelResults-or-None)."""
    P = np.asarray(P)
    plan, nc = _get_compiled(wa, wb)
    out = np.full((B, N_REG), 0.5, np.float32)
    if plan["A"] == 0:
        return out, None

    in_maps = make_in_maps(P, plan)
    res = bass_utils.run_bass_kernel_spmd(nc, in_maps, list(range(NCORES)), trace=trace)
    A = plan["A"]
    aidx = plan["aidx"]
    for c in range(NCORES):
        q = np.asarray(res.results[c]["outs"]).astype(np.float32)  # [Apad, BL]
        # device cast is round-to-nearest-even w/ saturation (probed on HW)
        s_core = (q - 128.0) * (1.0 / 254.0) + 0.5
        out[c * BL : (c + 1) * BL, aidx] = s_core[:A].T
    return out, res


def kernel(P, wa, wb):
    out, _ = run(P, wa, wb, trace=False)
    return out


# revision 25
# speedup vs baseline: 1.0887x; 1.0887x over previous
"""Trainium2 Bass kernel for nn_CircuitModel (soft sequential XOR circuit).

Math: with u = 1 - 2*s (s = register value), soft-XOR becomes a pure product:
    u_new = u_a * u_b,   u_x = -tanh(2*clip(p, -2, 2))
Magnitudes and signs decouple and both evolve *linearly*, so the whole
64-step scan unrolls at build time (wa/wb known when kernel() is called):
    log|u_final[j]| = sum_k C[j,k] * ln(tanh(2|p_k|))   (k = used (i,t) cell)
    sign(u_final[j]) = (-1)^( sum_k C[j,k] * [p_k > 0]  mod 2 )
    (parity of C.v == parity of D.v for D = C mod 2)
Registers whose tree hits the t=0 init state (u=0) or has leaf count
>= 4096 (magnitude underflow) output exactly 0.5.

Sharding: pure batch-parallel, 512 batch per core. The only per-exec
input is the signed bf16 value of each used (i,t) cell:
    ap = p as bf16 [128, nchunks, 512]   (a pure re-layout of P's numbers)
The wiring matrices cw (counts, bf16) and dw (parity, fp8) are baked into
the NEFF as Const tensors (loaded to device DRAM once at model load, not
per execution). Signs v = [p > 0] and |t| are computed on device (both on
DVE), so neither is transferred. Output is uint8 -- well inside the 2e-2
gate (see decode note below).

Device per core / rep:
  phase A (ACT table exp_and_others):
    t = tanh(2*ap) [ACT, signed]
    v = [ap > 0] fp8 {0,1} [DVE]        ta = max(-t, t) [DVE]
    (both on DVE: GpSimd's fp8-out tensor_scalar traps to a ~20us/half
     software handler on real HW)
    Y = D^T v: fp8 DoubleRow matmuls (2 chunks/pass; counts exact in f32)
    parity of Y via round-to-nearest magic (DVE):
        g = Y/2 + 2^23; h = g - 2^23 (= RN(Y/2)); d = Y/2 - h in {0,+-1/2}
        b = [d != 0] - 1/2  in {+-1/2} (bf16 exact)
  phase B (ACT table natural_log_exp_and_others):
    l = ln(ta) [ACT]; L = C^T l: bf16 matmuls
    e = exp(L) [ACT]; y = b*e [DVE]; q = uint8(y*254 + 128) [DVE] -> D2H
    (phase-B tail is per-m-tile: each 128-register tile's exp/y/q/out-DMA
     pipelines behind its last matmul)
Host decodes s = (q - 128)/254 + 0.5; the device's float->uint8 cast is
round-to-nearest-even with saturation (probed on HW), so quantization
error is <= 1/508.
"""

import sys
from contextlib import ExitStack

import numpy as np

sys.path.insert(0, "/opt/trn_rl_repo")

import concourse.mybir as mybir  # noqa: E402
import concourse.tile as tile  # noqa: E402
from concourse import bacc, bass_utils  # noqa: E402

N_IN = 256
N_REG = 256
T = 64
B = 4096
NCORES = 8
BL = B // NCORES  # 512 batch per core
W_CAP = 4096  # leaf-count threshold beyond which u underflows to 0 -> s = 0.5

AF = mybir.ActivationFunctionType
ALU = mybir.AluOpType
MAGIC = float(1 << 23)  # 2^23: fp32 round-to-nearest-integer magic


class _Bacc(bacc.Bacc):
    """Bacc whose ACT-table chooser never picks the exp-less 'natural_log'
    set: Ln then resolves to 'natural_log_exp_and_others', so the Ln pass
    and the final Exp share one table load instead of two."""

    def insert_act_table_loads(self):
        import bass_rust
        from concourse.hw_specs import get_activation_tables

        has_activation = any(
            isinstance(i, mybir.InstActivation)
            for b in self.main_func.blocks
            for i in b.instructions
        )
        if not has_activation:
            return
        tables = [
            (name, (set() if name == "natural_log" else funcs))
            for name, funcs in get_activation_tables(self.m.arch).items()
        ]
        bass_rust.insert_act_table_loads(self, tables)


def _unroll(wa, wb):
    """Exact symbolic unroll of the 64-step recurrence.

    Returns (C counts int64 [N_REG, N_IN*T] saturating, Z bool: u == 0
    exactly because the tree reaches the init state)."""
    NC = N_IN * T
    C = np.zeros((N_REG, NC), np.int64)
    Z = np.ones(N_REG, bool)
    wa = np.asarray(wa).astype(np.int64)
    wb = np.asarray(wb).astype(np.int64)
    for t in range(T):
        nC = np.zeros_like(C)
        nZ = np.zeros(N_REG, bool)
        for src in (wa, wb):
            is_x = src < N_IN
            xrows = np.nonzero(is_x)[0]
            nC[xrows, src[xrows] * T + t] += 1
            rrows = np.nonzero(~is_x)[0]
            ri = src[rrows] - N_IN
            nC[rrows] += C[ri]
            nZ[rrows] |= Z[ri]
        np.minimum(nC, 1 << 20, out=nC)
        C, Z = nC, nZ
    return C, Z


def _build_plan(wa, wb):
    C, Z = _unroll(wa, wb)
    W = C.sum(1)
    alive = (~Z) & (W < W_CAP)
    aidx = np.nonzero(alive)[0]
    A = int(len(aidx))
    if A == 0:
        return {"A": 0, "aidx": aidx}
    Ca = C[aidx]
    used = (Ca != 0).any(0)
    cols = np.nonzero(used)[0]  # flattened (i*T + t) indices of used cells
    ncols = int(len(cols))
    nchunks = (ncols + 127) // 128
    nslots = nchunks * 128
    # pad slots duplicate the first used cell; their C columns stay zero
    slot_cols = np.concatenate([cols, np.full(nslots - ncols, cols[0], np.int64)])
    slot_i = slot_cols // T
    slot_t = slot_cols % T

    n_mt = (A + 127) // 128
    Apad = n_mt * 128
    # lhsT layout [slot-in-chunk (K), chunk * Apad + alive-row (M)]
    cw = np.zeros((128, nchunks * Apad), np.float32)
    dw = np.zeros((128, nchunks * Apad), np.float32)
    for s in range(ncols):
        ci = slot_cols[s]
        c, k = divmod(s, 128)
        cw[k, c * Apad : c * Apad + A] = Ca[:, ci]
        dw[k, c * Apad : c * Apad + A] = Ca[:, ci] % 2
    return {
        "A": A,
        "aidx": aidx,
        "slot_i": slot_i,
        "slot_t": slot_t,
        "nchunks": nchunks,
        "n_mt": n_mt,
        "Apad": Apad,
        "cw": cw,
        "dw": dw,
    }


def _build_nc(plan, reps=1, loop=1):
    """reps: python-unrolled body repetitions (for slope benchmarking).
    loop: hardware For_i trip count around the body."""
    f32 = mybir.dt.float32
    bf16 = mybir.dt.bfloat16
    f8 = mybir.dt.float8e4
    u8 = mybir.dt.uint8
    DR = mybir.MatmulPerfMode.DoubleRow
    nchunks, n_mt, Apad = plan["nchunks"], plan["n_mt"], plan["Apad"]
    bfnp = mybir.dt.np(bf16)
    f8np = mybir.dt.np(f8)

    nc = _Bacc("TRN2", debug=False)
    # [slot-in-chunk, chunk, batch], packed on host while sharding (signed p)
    ap_d = nc.dram_tensor("ap_used", [128, nchunks, BL], bf16, kind="ExternalInput")
    # wiring weights: Const tensors inside the NEFF (no per-exec H2D)
    cw_d = nc.inline_tensor(plan["cw"].astype(bfnp), name="cw_const")
    dw_d = nc.inline_tensor(plan["dw"].astype(f8np), name="dw_const")
    out_d = nc.dram_tensor("outs", [Apad, BL], u8, kind="ExternalOutput")

    # split chunks in two halves so ACT starts while the 2nd DMA flies
    h0 = max(1, nchunks // 2)
    halves = [(0, h0), (h0, nchunks)] if nchunks > 1 else [(0, nchunks)]

    with tile.TileContext(nc) as tc, ExitStack() as ctx:
        pool = ctx.enter_context(tc.tile_pool(name="pool", bufs=1))
        tmp = ctx.enter_context(tc.tile_pool(name="tmp", bufs=2))
        mps = ctx.enter_context(tc.tile_pool(name="mps", bufs=1, space="PSUM"))

        # weights go over the ACT-engine HWDGE queue so the batch input's
        # DMAs (SP queue) aren't queued behind them
        cw_s = pool.tile([128, nchunks * Apad], bf16)
        nc.scalar.dma_start(cw_s[:], cw_d[:])
        dw_s = pool.tile([128, nchunks * Apad], f8)
        nc.scalar.dma_start(dw_s[:], dw_d[:])
        cwv = cw_s.rearrange("k (c a) -> k c a", a=Apad)
        dwv = dw_s.rearrange("k (c a) -> k c a", a=Apad)

        # all loop tiles hoisted out of the rep body: per-rep tile
        # alloc/release sync ops were a visible chunk of the HW slop.
        # Data hazards across reps are still tracked per-access on the
        # reused handles.
        nb = min(2, max(1, reps))
        T_ap = [pool.tile([128, nchunks * BL], bf16, name=f"ap{i}") for i in range(nb)]
        T_t = [pool.tile([128, nchunks * BL], bf16, name=f"t{i}") for i in range(nb)]
        T_ta = [pool.tile([128, nchunks * BL], bf16, name=f"ta{i}") for i in range(nb)]
        T_v = [pool.tile([128, nchunks * BL], f8, name=f"v{i}") for i in range(nb)]
        T_b = [pool.tile([128, n_mt * BL], bf16, name=f"b{i}") for i in range(nb)]
        T_l = [pool.tile([128, nchunks * BL], bf16, name=f"l{i}") for i in range(nb)]
        T_g = [tmp.tile([128, n_mt * BL], f32, name=f"p_g{i}", tag=f"p_g{i}") for i in range(nb)]
        T_h = [tmp.tile([128, n_mt * BL], f32, name=f"p_h{i}", tag=f"p_h{i}") for i in range(nb)]
        T_d = [tmp.tile([128, n_mt * BL], bf16, name=f"p_d{i}", tag=f"p_d{i}") for i in range(nb)]
        T_e = [tmp.tile([128, n_mt * BL], bf16, name=f"p_e{i}", tag=f"p_e{i}") for i in range(nb)]
        T_y = [tmp.tile([128, n_mt * BL], bf16, name=f"p_y{i}", tag=f"p_y{i}") for i in range(nb)]
        T_q = [tmp.tile([128, n_mt * BL], u8, name=f"p_q{i}", tag=f"p_q{i}") for i in range(nb)]
        T_Y = [mps.tile([128, n_mt * BL], f32, name=f"Yp{i}") for i in range(nb)]
        T_L = [mps.tile([128, n_mt * BL], f32, name=f"Lp{i}") for i in range(nb)]

        def body():
            for r in range(reps):
                i = r % nb
                apv_d = ap_d.rearrange("k c b -> k (c b)")
                ap_s = T_ap[i]
                t_s = T_t[i]
                ta_s = T_ta[i]
                v_s = T_v[i]
                for c0, c1 in halves:
                    sl = slice(c0 * BL, c1 * BL)
                    nc.sync.dma_start(ap_s[:, sl], apv_d[:, sl])
                    # ---- phase A: tanh (table: exp_and_others) ----
                    nc.scalar.activation(t_s[:, sl], ap_s[:, sl], AF.Tanh, scale=2.0)
                    # signs from raw p. NOT on GpSimd: Pool's is_gt
                    # tensor_scalar traps to a ~20us/half software handler
                    # on real HW regardless of dtype (cost model: 2.2us).
                    # Interleaved per half so the long v op never blocks the
                    # next |t| (critical for Ln) in the DVE program order.
                    nc.vector.tensor_scalar(
                        v_s[:, sl], ap_s[:, sl], 0.0, None, ALU.is_gt
                    )
                    # |t| on DVE: max(-t, t)
                    nc.vector.scalar_tensor_tensor(
                        ta_s[:, sl], t_s[:, sl], -1.0, t_s[:, sl], ALU.mult, ALU.max
                    )
                vv = v_s.rearrange("k (c b) -> k c b", b=BL)
                tav = ta_s.rearrange("k (c b) -> k c b", b=BL)

                # ---- parity counts: Y = D^T v (fp8 DoubleRow, 2 chunks
                # per pass: halves the cold-clock PE serial chain vs bf16)
                Y_ps = T_Y[i]
                npair = nchunks // 2
                for c in range(npair):
                    for mt in range(n_mt):
                        nc.tensor.matmul(
                            Y_ps[:, mt * BL : (mt + 1) * BL],
                            dwv[:, 2 * c : 2 * c + 2, mt * 128 : (mt + 1) * 128],
                            vv[:, 2 * c : 2 * c + 2, :],
                            start=(c == 0),
                            stop=(c == npair - 1 and nchunks % 2 == 0),
                            perf_mode=DR,
                        )
                if nchunks % 2:  # odd tail chunk: plain fp8 matmul
                    c = nchunks - 1
                    for mt in range(n_mt):
                        nc.tensor.matmul(
                            Y_ps[:, mt * BL : (mt + 1) * BL],
                            dwv[:, c, mt * 128 : (mt + 1) * 128],
                            vv[:, c, :],
                            start=(nchunks == 1),
                            stop=True,
                        )
                # parity: b = [Y odd] - 1/2, exact via fp32 magic round
                g_t = T_g[i]
                nc.vector.tensor_scalar(g_t[:], Y_ps[:], 0.5, MAGIC, ALU.mult, ALU.add)
                h_t = T_h[i]
                nc.vector.tensor_scalar(h_t[:], g_t[:], MAGIC, None, ALU.subtract)
                d_t = T_d[i]
                nc.vector.scalar_tensor_tensor(
                    d_t[:], Y_ps[:], 0.5, h_t[:], ALU.mult, ALU.subtract
                )
                b_t = T_b[i]
                nc.vector.tensor_scalar(
                    b_t[:], d_t[:], 0.0, 0.5, ALU.not_equal, ALU.subtract
                )

                # ---- phase B: ln + counts matmul + exp (table: nat_log_exp)
                l_s = T_l[i]
                L_ps = T_L[i]
                lv = l_s.rearrange("k (c b) -> k c b", b=BL)
                e_t = T_e[i]
                y_t = T_y[i]
                q_t = T_q[i]
                for c0, c1 in halves[:-1]:
                    sl = slice(c0 * BL, c1 * BL)
                    nc.scalar.activation(l_s[:, sl], ta_s[:, sl], AF.Ln)
                    for c in range(c0, c1):
                        for mt in range(n_mt):
                            nc.tensor.matmul(
                                L_ps[:, mt * BL : (mt + 1) * BL],
                                cwv[:, c, mt * 128 : (mt + 1) * 128],
                                lv[:, c, :],
                                start=(c == 0),
                                stop=False,
                            )
                # last half mt-major: exp/y/q/out-DMA pipeline per m-tile
                c0, c1 = halves[-1]
                sl = slice(c0 * BL, c1 * BL)
                nc.scalar.activation(l_s[:, sl], ta_s[:, sl], AF.Ln)
                for mt in range(n_mt):
                    mb = slice(mt * BL, (mt + 1) * BL)
                    for c in range(c0, c1):
                        nc.tensor.matmul(
                            L_ps[:, mb],
                            cwv[:, c, mt * 128 : (mt + 1) * 128],
                            lv[:, c, :],
                            start=(c == 0 and c0 == 0),
                            stop=(c == nchunks - 1),
                        )
                    nc.scalar.activation(e_t[:, mb], L_ps[:, mb], AF.Exp)
                    nc.vector.tensor_tensor(y_t[:, mb], b_t[:, mb], e_t[:, mb], ALU.mult)
                    nc.vector.tensor_scalar(
                        q_t[:, mb], y_t[:, mb], 254.0, 128.0, ALU.mult, ALU.add
                    )
                    nc.sync.dma_start(
                        out_d[mt * 128 : (mt + 1) * 128, :], q_t[:, mb]
                    )

        if loop > 1:
            with tc.For_i(0, loop):
                body()
        else:
            body()

    nc.compile()
    return nc


_CACHE = {}


def _get_compiled(wa, wb):
    key = (np.asarray(wa).tobytes(), np.asarray(wb).tobytes())
    if key not in _CACHE:
        plan = _build_plan(wa, wb)
        nc = _build_nc(plan) if plan["A"] > 0 else None
        _CACHE[key] = (plan, nc)
    return _CACHE[key]


def make_in_maps(P, plan):
    """Pack per-core input: signed p of used cells, bf16 [128, nchunks, BL]."""
    bf = mybir.dt.np(mybir.dt.bfloat16)
    nchunks = plan["nchunks"]
    sel_all = P[plan["slot_i"], :, plan["slot_t"]]  # [nslots, B] f32
    sel_all = sel_all.reshape(nchunks, 128, B).transpose(1, 0, 2)  # [128,c,B]
    sel_all = sel_all.astype(bf)
    return [
        {"ap_used": np.ascontiguousarray(sel_all[:, :, c * BL : (c + 1) * BL])}
        for c in range(NCORES)
    ]


def run(P, wa, wb, trace=False):
    """Returns (out [B, N_REG] float32, BassKernelResults-or-None)."""
    P = np.asarray(P)
    plan, nc = _get_compiled(wa, wb)
    out = np.full((B, N_REG), 0.5, np.float32)
    if plan["A"] == 0:
        return out, None

    in_maps = make_in_maps(P, plan)
    res = bass_utils.run_bass_kernel_spmd(nc, in_maps, list(range(NCORES)), trace=trace)
    A = plan["A"]
    aidx = plan["aidx"]
    for c in range(NCORES):
        q = np.asarray(res.results[c]["outs"]).astype(np.float32)  # [Apad, BL]
        # device cast is round-to-nearest-even w/ saturation (probed on HW)
        s_core = (q - 128.0) * (1.0 / 254.0) + 0.5
        out[c * BL : (c + 1) * BL, aidx] = s_core[:A].T
    return out, res


def kernel(P, wa, wb):
    out, _ = run(P, wa, wb, trace=False)
    return out
